# revision 22
# baseline (speedup 1.0000x reference)
"""MoE layer (8 experts, top-2 sigmoid routing, SwiGLU experts + shared expert)
on 8 TRN2 NeuronCores.

Strategy (expert-parallel, host-side token dispatch):
  - Router (sigmoid(x @ gate_w.T), top-2, weight normalization) is tiny
    (~50 MFLOP) and runs on the host; it determines the all-to-all dispatch.
  - Core c owns expert c: it gets the tokens routed to expert c (gathered and
    zero-padded to a common capacity m_pad) plus expert c's Wi/Wo.
  - The shared expert is data-parallel: core c also processes tokens
    [c*512, (c+1)*512) with the (replicated) shared weights.
  - Device kernel computes the two SwiGLU MLP passes in bf16 (fp32 PSUM
    accumulation), feature-major layout (features on partitions, tokens on the
    free dim) so no on-device transposes are needed.
  - Host combine: out[t] = shared_out[t] + sum_e cw[e,t] * expert_out[e][t]
    (the combine weights are applied on the host during the scatter-add).

Perf notes (v2, from NTFF trace analysis):
  - The PE matmul stream is already back-to-back at the streaming rate; the
    recoverable time was (a) DMA *issue* rate — each DMA_DIRECT2D costs
    ~750 ns on the Sync engine, and the old kernel issued ~60 input DMAs
    (~50 us of issue, gating the lead-in) — all inputs are now host-packed
    into 2D-contiguous [128, X] tensors so each logical load is ONE issue;
    (b) warm-up overshoot — 8 matmuls (~4 us at the cold 1/2-rate clock)
    cover the HAM activity window and the lead-in DMA, 16 overshot;
    (c) output DMA bytes — outputs go back bf16 (host upcasts), halving the
    post-last-matmul drain.
"""

from contextlib import ExitStack

import ml_dtypes
import numpy as np

import concourse.tile as tile
from concourse import bacc, mybir
from concourse.bass_utils import run_bass_kernel_spmd

E, TOPK, H, I = 8, 2, 768, 1152
I2 = 2 * I
T = 4096
N_CORES = 8
TS = T // N_CORES  # shared-expert tokens per core
P = 128
KH = H // P    # 6 contraction tiles over H
KI = I // P    # 9 contraction tiles over I
BF16 = mybir.dt.bfloat16
F32 = mybir.dt.float32
MAXN = 512     # max tokens per matmul chunk (one fp32 PSUM bank)
# HAM warm-up matmuls. MUST give >=3.4us of continuous cold-clock busy
# (8 was borderline: one run flipped warm during warm-up, the next ran its
# first ~10 real matmuls at the cold half-rate clock, +2us).
N_WARM = 10

_BUILD_CACHE: dict = {}
LAST_RESULTS = None  # BassKernelResults of the most recent device run
USE_SILU = True  # native ACT Silu on HW; set False for CoreSim (not implemented there)


def _ensure_axon_ntff_hook():
    """This image's `antenv` lacks the `axon_hooks` module that
    run_bass_kernel_spmd imports when NTFF tracing is requested (BASS_TRACE=1).
    Install an equivalent shim so profiling works instead of crashing."""
    try:
        import antenv.axon_hooks  # noqa: F401
        return
    except ImportError:
        pass
    import sys
    import types
    try:
        import antenv
    except ImportError:
        return
    mod = types.ModuleType("antenv.axon_hooks")
    holder = {"hook": None}
    mod.set_axon_ntff_profile_hook = lambda h: holder.__setitem__("hook", h)
    mod.get_axon_ntff_profile_hook = lambda: holder["hook"]
    sys.modules["antenv.axon_hooks"] = mod
    antenv.axon_hooks = mod
    so_path = "/opt/axon/libaxon_pjrt.so"
    try:
        import os
        if os.path.exists(so_path):
            from trn_agent_boot.trn_boot import _ntff_profile_via_ctypes
            hook = _ntff_profile_via_ctypes(so_path)
            if hook is not None:
                mod.set_axon_ntff_profile_hook(hook)
    except Exception:
        pass  # hook stays None; bass_utils logs a warning and skips tracing


def _chunk_sizes(m: int) -> list[int]:
    """Split m into ceil(m/512) chunks, biggest first and a small (but
    >=192-col, so LDWEIGHTS stays hidden under streaming) LAST chunk — the
    final chunk's Wo chains + copy + output DMA are the post-matmul tail."""
    n = -(-m // MAXN)
    if n == 1:
        return [m]
    last = max(192, m - (n - 1) * MAXN)
    base, rem = divmod(m - last, n - 1)
    return [base + 1] * rem + [base] * (n - 1 - rem) + [last]


def _build(m_pad: int):
    nc = bacc.Bacc("TRN2", target_bir_lowering=False, debug=False,
                   num_devices=N_CORES)

    # All inputs are host-packed [128, X] row-major so every load below is a
    # single 2D-contiguous DMA (one DMA_DIRECT2D issue each — the issue rate
    # on the Sync engine, ~750ns/DMA, was the old kernel's lead-in gate).
    #   xs[p, kt*TS + c]        = x[kt*P + p, shared token c]       (bf16)
    #   xe[p, kt*m_pad + c]     = x[kt*P + p, expert token c]       (bf16)
    #   wi[p, kt*I2 + j]        = Wi[kt*P + p, j]                   (bf16)
    #   wo[p, kt*H + j]         = Wo[kt*P + p, j]                   (bf16)
    #   swo[p, kt*H + j]        = shared_Wo.T[kt*P + p, j]          (bf16)
    #   swi[p, ft*(2H) + s*H + j] = shared_Wi.T[j_tile] pair-interleaved:
    #       s=0 -> A-half f-tile ft, s=1 -> B-half f-tile ft (see _pack_swi)
    xs = nc.dram_tensor("xs", [P, KH * TS], BF16, kind="ExternalInput").ap()
    xe = nc.dram_tensor("xe", [P, KH * m_pad], BF16, kind="ExternalInput").ap()
    wi = nc.dram_tensor("wi", [P, KH * I2], BF16, kind="ExternalInput").ap()
    wo = nc.dram_tensor("wo", [P, KI * H], BF16, kind="ExternalInput").ap()
    swi = nc.dram_tensor("swi", [P, KI * 2 * H], BF16, kind="ExternalInput").ap()
    swo = nc.dram_tensor("swo", [P, KI * H], BF16, kind="ExternalInput").ap()
    ye = nc.dram_tensor("ye", [H, m_pad], BF16, kind="ExternalOutput").ap()
    ys = nc.dram_tensor("ys", [H, TS], BF16, kind="ExternalOutput").ap()

    # Raw (un-tracked, never-written) SBUF region for the HAM warm-up
    # matmuls: no memset and no dependencies, so the PE starts warming the
    # clock gate immediately after its preamble — maximum margin for the
    # free-running HAM window to flip before the real matmuls begin.
    # The garbage values it computes go to a scratch DRAM sink.
    warm_sb = nc.alloc_sbuf_tensor("warm_raw", [P, MAXN], BF16).ap()

    with ExitStack() as ctx:
        tc = ctx.enter_context(tile.TileContext(nc))
        wpool = ctx.enter_context(tc.tile_pool(name="weights", bufs=1))
        apool = ctx.enter_context(tc.tile_pool(name="act", bufs=3))
        spool = ctx.enter_context(tc.tile_pool(name="silu", bufs=4))
        # yt slots are freed by their output DMA's completion; 3 bufs made a
        # Wo-phase matmul chain wait on the SWDGE drain once per trace
        ypool = ctx.enter_context(tc.tile_pool(name="y", bufs=6))
        # all 8 PSUM banks in one pool; the warm-up tiles share the "ps" tag
        # so their 2 banks recycle into the working set after the lead-in
        psum = ctx.enter_context(tc.tile_pool(name="psum", bufs=8, space="PSUM"))

        # Warm-up matmuls on scratch data fill the otherwise-idle PE during
        # the DMA lead-in: the HAM clock gate sees a busy window and
        # un-throttles before the real matmuls start.
        warm_sink = nc.dram_tensor("warm_sink", [P, MAXN], F32).ap()
        wps = [psum.tile([P, MAXN], F32, tag="ps", name=f"wps{i}")
               for i in range(2)]
        for i in range(N_WARM):
            nc.tensor.matmul(wps[i % 2], lhsT=warm_sb[:, :P], rhs=warm_sb[:],
                             start=True, stop=True)
        warm_out = ypool.tile([P, MAXN], F32, tag="y", name="warm_out")
        nc.vector.tensor_copy(warm_out[:], wps[1])
        nc.gpsimd.dma_start(warm_sink[:], warm_out[:])

        # shared-expert job first: its x slice and the first shared-Wi pair
        # are the smallest loads, so the PE starts earliest and the (larger)
        # expert weight/token streams hide behind the shared job's compute.
        # Each HWDGE queue moves ~160 GB/s, so the first chain's inputs are
        # split across queues (xs in halves, first swi pair in halves) —
        # they land ~1.7us earlier than single-queue transfers would.
        # Issue order follows the first chain's consumption order: the ps_a
        # chain reads swiP0-A + xs k-tiles 0..5, then ps_b reads swiP0-B —
        # so (xs lo-half, A0, xs hi-half, B0) lets the chain start as soon
        # as the first pieces land instead of after the whole lead set.
        xs_t = wpool.tile([P, KH, TS], BF16, tag="xs", name="xs")
        xs3 = xs.rearrange("p (o f) -> p o f", o=KH)
        swi_p = [wpool.tile([P, 2, H], BF16, tag=f"swiP{ft}", name=f"swiP{ft}")
                 for ft in range(KI)]

        def swi_src(ft):
            return (swi[:, ft * 2 * H:(ft + 1) * 2 * H]
                    .rearrange("p (s f) -> p s f", s=2))

        # (Splitting EVERY pair into halves was tried and hurt: the extra
        # early DMAs compete with the critical xs hi-half and stalled the
        # first chain mid-stream for ~2.8us.)
        nc.sync.dma_start(xs_t[:, :KH // 2], xs3[:, :KH // 2])
        nc.sync.dma_start(swi_p[0][:, 0], swi_src(0)[:, 0])
        nc.sync.dma_start(xs_t[:, KH // 2:], xs3[:, KH // 2:])
        nc.sync.dma_start(swi_p[0][:, 1], swi_src(0)[:, 1])
        for ft in range(1, KI):
            nc.sync.dma_start(swi_p[ft][:], swi_src(ft))

        named = {}  # late-bound tiles for the expert job + shared Wo

        # accessors: (ft|ht, kt) -> lhsT AP; x: (kt) -> rhs AP
        sh = dict(
            x=lambda kt: xs_t[:, kt],
            wa=lambda ft, kt: swi_p[ft][:, 0, kt * P:(kt + 1) * P],
            wb=lambda ft, kt: swi_p[ft][:, 1, kt * P:(kt + 1) * P],
            wo=lambda ht, kt: named["swo"][:, kt, ht * P:(ht + 1) * P],
        )
        ex = dict(
            x=lambda kt: named["xe"][:, kt],
            wa=lambda ft, kt: named["wi"][:, kt, ft * P:(ft + 1) * P],
            wb=lambda ft, kt: named["wi"][:, kt, I + ft * P:I + (ft + 1) * P],
            wo=lambda ht, kt: named["wo"][:, kt, ht * P:(ht + 1) * P],
        )

        # (accessors, y_dram, chunk_off, chunk_sz, silu_on_first)
        chunks = []
        for acc, yd, m, sfirst in ((sh, ys, TS, True), (ex, ye, m_pad, False)):
            off = 0
            for sz in _chunk_sizes(m):
                chunks.append((acc, yd, off, sz, sfirst))
                off += sz

        def emit_wi(c):
            acc, yd, off, sz, sfirst = chunks[c]
            act = apool.tile([P, KI, MAXN], BF16, tag="act", name="act")[:, :, :sz]
            # the silu-input half's chain runs FIRST and its silu is emitted
            # immediately after it — the bank frees a full chain earlier
            # (the periodic ~432ns bank-wait stalls were shared-phase only,
            # where silu used to sit after the second chain)
            for ft in range(KI):
                ws, wm = (("wa", "wb") if sfirst else ("wb", "wa"))
                ps_s = psum.tile([P, MAXN], F32, tag="ps", name="ps_s")[:, :sz]
                for kt in range(KH):
                    nc.tensor.matmul(ps_s, lhsT=acc[ws](ft, kt),
                                     rhs=acc["x"](kt)[:, off:off + sz],
                                     start=(kt == 0), stop=(kt == KH - 1))
                sl = spool.tile([P, MAXN], F32, tag="silu", name="sl")[:, :sz]
                if USE_SILU:
                    # act = silu(s) * m: one ACT op + one DVE mul
                    nc.scalar.activation(sl, ps_s,
                                         mybir.ActivationFunctionType.Silu)
                else:
                    # CoreSim fallback: silu(s) = s * sigmoid(s)
                    tmp = spool.tile([P, MAXN], F32, tag="silu2",
                                     name="tmp")[:, :sz]
                    nc.scalar.activation(sl, ps_s,
                                         mybir.ActivationFunctionType.Sigmoid)
                    nc.vector.tensor_mul(tmp, sl, ps_s)
                    sl = tmp
                ps_m = psum.tile([P, MAXN], F32, tag="ps", name="ps_m")[:, :sz]
                for kt in range(KH):
                    nc.tensor.matmul(ps_m, lhsT=acc[wm](ft, kt),
                                     rhs=acc["x"](kt)[:, off:off + sz],
                                     start=(kt == 0), stop=(kt == KH - 1))
                nc.vector.tensor_mul(act[:, ft, :], sl, ps_m)
            return act

        def emit_wo(c, act, last=False):
            acc, yd, off, sz, sfirst = chunks[c]
            ydst = yd.rearrange("(o p) m -> p o m", p=P)
            for ht in range(KH):
                ps_y = psum.tile([P, MAXN], F32, tag="ps", name="ps_y")[:, :sz]
                for kt in range(KI):
                    nc.tensor.matmul(ps_y, lhsT=acc["wo"](ht, kt),
                                     rhs=act[:, kt, :],
                                     start=(kt == 0), stop=(kt == KI - 1))
                yt = ypool.tile([P, MAXN], BF16, tag="y", name="yt")[:, :sz]
                # These copies interleave (software-pipelined) with the NEXT
                # chunk's silu/mul PSUM consumers on strict-FIFO engine
                # queues — a copy queued ahead of a silu delayed the PSUM
                # bank hand-back every 4th f-tile (432ns stall each).
                # Alternating ACT/DVE halves each queue's copy burden; the
                # last chunk's copies all go to the (by-then idle) DVE.
                if last or ht % 2:
                    nc.vector.tensor_copy(yt, ps_y)
                else:
                    nc.scalar.copy(yt, ps_y)
                dma_eng = nc.sync if last else nc.gpsimd
                dma_eng.dma_start(ydst[:, ht, off:off + sz], yt)

        # software pipeline: Wi(c+1) is emitted before Wo(c) so the PE always
        # has independent matmul work while ACT/DVE finish chunk c's SwiGLU.
        # Remaining loads are emitted right after the first chunk's Wi.
        n = len(chunks)
        acts = [None] * n
        acts[0] = emit_wi(0)
        t = wpool.tile([P, KI, H], BF16, tag="swo", name="swo")
        nc.sync.dma_start(t[:], swo.rearrange("p (o f) -> p o f", o=KI))
        named["swo"] = t
        t = wpool.tile([P, KH, m_pad], BF16, tag="xe", name="xe")
        nc.sync.dma_start(t[:], xe.rearrange("p (o f) -> p o f", o=KH))
        named["xe"] = t
        t = wpool.tile([P, KH, I2], BF16, tag="wi", name="wi")
        nc.sync.dma_start(t[:], wi.rearrange("p (o f) -> p o f", o=KH))
        named["wi"] = t
        t = wpool.tile([P, KI, H], BF16, tag="wo", name="wo")
        nc.sync.dma_start(t[:], wo.rearrange("p (o f) -> p o f", o=KI))
        named["wo"] = t
        # shared Wo BEFORE the first expert Wi: its weights are already
        # resident, so the PE never head-of-line blocks on the expert weight
        # stream (an idle window >3.4us would re-throttle the HAM clock gate)
        emit_wo(0, acts[0])
        if n > 1:
            acts[1] = emit_wi(1)
            for c in range(2, n):
                acts[c] = emit_wi(c)
                emit_wo(c - 1, acts[c - 1])
            emit_wo(n - 1, acts[n - 1], last=True)

    nc.compile()
    return nc


def _pack_rows(a: np.ndarray) -> np.ndarray:
    """(ktiles*P, F) f32/bf16 -> packed [P, ktiles*F] (row p holds each
    k-tile's row p, concatenated k-tile-major)."""
    kt = a.shape[0] // P
    return np.ascontiguousarray(
        a.reshape(kt, P, a.shape[1]).transpose(1, 0, 2).reshape(P, -1))


def _pack_swi(swiT: np.ndarray) -> np.ndarray:
    """shared_Wi.T (H, 2I) -> [P, KI*2*H]: per f-tile pair (A_ft, B_ft),
    each [P, H] with swi[p, ft, s, kt*P + c] = swiT[kt*P + p, (s*I) + ft*P + c]."""
    # (H, 2I) -> (KH, P, 2, KI, P): kt, p, s(half), ft, c
    r = swiT.reshape(KH, P, 2, KI, P)
    # -> (p, ft, s, kt, c)
    return np.ascontiguousarray(
        r.transpose(1, 3, 2, 0, 4).reshape(P, -1))


def _route(x, gate_w, correction_bias):
    logits = 1.0 / (1.0 + np.exp(-(x @ gate_w.T), dtype=np.float32))  # (T, E)
    sel = logits + correction_bias[None, :]
    order = np.argsort(-sel, axis=1, kind="stable")[:, :TOPK]  # ties -> low index
    w = np.take_along_axis(logits, order, axis=1)
    w = (w / w.sum(axis=1, keepdims=True)).astype(np.float32)
    return order, w


def kernel(**inputs) -> np.ndarray:
    x = np.asarray(inputs["x"], np.float32)
    gate_w = np.asarray(inputs["gate_w"], np.float32)
    bias = np.asarray(inputs["correction_bias"], np.float32)
    Wi = np.asarray(inputs["Wi"], np.float32)
    Wo = np.asarray(inputs["Wo"], np.float32)
    shared_Wi = np.asarray(inputs["shared_Wi"], np.float32)
    shared_Wo = np.asarray(inputs["shared_Wo"], np.float32)

    order, w = _route(x, gate_w, bias)

    idx_per_e, cw_per_e = [], []
    for e in range(E):
        mask = order == e  # (T, K)
        tok = mask.any(axis=1)
        rows = np.nonzero(tok)[0]
        kpos = np.argmax(mask[rows], axis=1)
        idx_per_e.append(rows)
        cw_per_e.append(w[rows, kpos].astype(np.float32))

    mx = max(len(r) for r in idx_per_e)
    m_pad = max(64, mx + (mx & 1))  # exact capacity, kept even for alignment

    bf = ml_dtypes.bfloat16
    xT = np.ascontiguousarray(x.T).astype(bf)        # (H, T) bf16
    swi_packed = _pack_swi(shared_Wi.T.astype(bf))   # [P, KI*2*H]
    swo_packed = _pack_rows(
        np.ascontiguousarray(shared_Wo.T).astype(bf))  # [P, KI*H]

    in_maps = []
    for c in range(N_CORES):
        rows = idx_per_e[c]
        xe = np.zeros((H, m_pad), bf)
        xe[:, :len(rows)] = xT[:, rows]
        in_maps.append({
            "xe": _pack_rows(xe),                            # [P, KH*m_pad]
            "wi": _pack_rows(Wi[c].astype(bf)),              # [P, KH*2I]
            "wo": _pack_rows(Wo[c].astype(bf)),              # [P, KI*H]
            "xs": _pack_rows(
                np.ascontiguousarray(xT[:, c * TS:(c + 1) * TS])),
            "swi": swi_packed,
            "swo": swo_packed,
        })

    if m_pad not in _BUILD_CACHE:
        _BUILD_CACHE[m_pad] = _build(m_pad)
    nc = _BUILD_CACHE[m_pad]

    _ensure_axon_ntff_hook()
    res = run_bass_kernel_spmd(nc, in_maps, list(range(N_CORES)))
    global LAST_RESULTS
    LAST_RESULTS = res

    out = np.zeros((T, H), np.float32)
    for c in range(N_CORES):
        r = res.results[c]
        out[c * TS:(c + 1) * TS] += r["ys"].astype(np.float32).T
        rows = idx_per_e[c]
        if len(rows):
            out[rows] += (r["ye"][:, :len(rows)].astype(np.float32).T
                          * cw_per_e[c][:, None])
    return out


# revision 23
# speedup vs baseline: 1.0095x; 1.0095x over previous
"""MoE layer (8 experts, top-2 sigmoid routing, SwiGLU experts + shared expert)
on 8 TRN2 NeuronCores.

Strategy (expert-parallel, host-side token dispatch):
  - Router (sigmoid(x @ gate_w.T), top-2, weight normalization) is tiny
    (~50 MFLOP) and runs on the host; it determines the all-to-all dispatch.
  - Core c owns expert c: it gets the tokens routed to expert c (gathered and
    zero-padded to a common capacity m_pad) plus expert c's Wi/Wo.
  - The shared expert is data-parallel: core c also processes tokens
    [c*512, (c+1)*512) with the (replicated) shared weights.
  - Device kernel computes the two SwiGLU MLP passes in bf16 (fp32 PSUM
    accumulation), feature-major layout (features on partitions, tokens on the
    free dim) so no on-device transposes are needed.
  - Host combine: out[t] = shared_out[t] + sum_e cw[e,t] * expert_out[e][t]
    (the combine weights are applied on the host during the scatter-add).

Perf notes (v2, from NTFF trace analysis):
  - The PE matmul stream is already back-to-back at the streaming rate; the
    recoverable time was (a) DMA *issue* rate — each DMA_DIRECT2D costs
    ~750 ns on the Sync engine, and the old kernel issued ~60 input DMAs
    (~50 us of issue, gating the lead-in) — all inputs are now host-packed
    into 2D-contiguous [128, X] tensors so each logical load is ONE issue;
    (b) warm-up overshoot — 8 matmuls (~4 us at the cold 1/2-rate clock)
    cover the HAM activity window and the lead-in DMA, 16 overshot;
    (c) output DMA bytes — outputs go back bf16 (host upcasts), halving the
    post-last-matmul drain.
"""

from contextlib import ExitStack

import ml_dtypes
import numpy as np

import concourse.tile as tile
from concourse import bacc, mybir
from concourse.bass_utils import run_bass_kernel_spmd

E, TOPK, H, I = 8, 2, 768, 1152
I2 = 2 * I
T = 4096
N_CORES = 8
TS = T // N_CORES  # shared-expert tokens per core
P = 128
KH = H // P    # 6 contraction tiles over H
KI = I // P    # 9 contraction tiles over I
BF16 = mybir.dt.bfloat16
F32 = mybir.dt.float32
MAXN = 512     # max tokens per matmul chunk (one fp32 PSUM bank)
# HAM warm-up matmuls. MUST give >=3.4us of continuous cold-clock busy
# (8 was borderline: one run flipped warm during warm-up, the next ran its
# first ~10 real matmuls at the cold half-rate clock, +2us).
N_WARM = 10

_BUILD_CACHE: dict = {}
LAST_RESULTS = None  # BassKernelResults of the most recent device run
USE_SILU = True  # native ACT Silu on HW; set False for CoreSim (not implemented there)


def _ensure_axon_ntff_hook():
    """This image's `antenv` lacks the `axon_hooks` module that
    run_bass_kernel_spmd imports when NTFF tracing is requested (BASS_TRACE=1).
    Install an equivalent shim so profiling works instead of crashing."""
    try:
        import antenv.axon_hooks  # noqa: F401
        return
    except ImportError:
        pass
    import sys
    import types
    try:
        import antenv
    except ImportError:
        return
    mod = types.ModuleType("antenv.axon_hooks")
    holder = {"hook": None}
    mod.set_axon_ntff_profile_hook = lambda h: holder.__setitem__("hook", h)
    mod.get_axon_ntff_profile_hook = lambda: holder["hook"]
    sys.modules["antenv.axon_hooks"] = mod
    antenv.axon_hooks = mod
    so_path = "/opt/axon/libaxon_pjrt.so"
    try:
        import os
        if os.path.exists(so_path):
            from trn_agent_boot.trn_boot import _ntff_profile_via_ctypes
            hook = _ntff_profile_via_ctypes(so_path)
            if hook is not None:
                mod.set_axon_ntff_profile_hook(hook)
    except Exception:
        pass  # hook stays None; bass_utils logs a warning and skips tracing


def _chunk_sizes(m: int) -> list[int]:
    """Split m into ceil(m/512) chunks, biggest first and a small (but
    >=192-col, so LDWEIGHTS stays hidden under streaming) LAST chunk — the
    final chunk's Wo chains + copy + output DMA are the post-matmul tail."""
    n = -(-m // MAXN)
    if n == 1:
        return [m]
    last = max(192, m - (n - 1) * MAXN)
    base, rem = divmod(m - last, n - 1)
    return [base + 1] * rem + [base] * (n - 1 - rem) + [last]


def _build(m_pad: int):
    nc = bacc.Bacc("TRN2", target_bir_lowering=False, debug=False,
                   num_devices=N_CORES)

    # All inputs are host-packed [128, X] row-major so every load below is a
    # single 2D-contiguous DMA (one DMA_DIRECT2D issue each — the issue rate
    # on the Sync engine, ~750ns/DMA, was the old kernel's lead-in gate).
    #   xs[p, kt*TS + c]        = x[kt*P + p, shared token c]       (bf16)
    #   xe[p, kt*m_pad + c]     = x[kt*P + p, expert token c]       (bf16)
    #   wi[p, kt*I2 + j]        = Wi[kt*P + p, j]                   (bf16)
    #   wo[p, kt*H + j]         = Wo[kt*P + p, j]                   (bf16)
    #   swo[p, kt*H + j]        = shared_Wo.T[kt*P + p, j]          (bf16)
    #   swi[p, ft*(2H) + s*H + j] = shared_Wi.T[j_tile] pair-interleaved:
    #       s=0 -> A-half f-tile ft, s=1 -> B-half f-tile ft (see _pack_swi)
    xs = nc.dram_tensor("xs", [P, KH * TS], BF16, kind="ExternalInput").ap()
    xe = nc.dram_tensor("xe", [P, KH * m_pad], BF16, kind="ExternalInput").ap()
    wi = nc.dram_tensor("wi", [P, KH * I2], BF16, kind="ExternalInput").ap()
    wo = nc.dram_tensor("wo", [P, KI * H], BF16, kind="ExternalInput").ap()
    swi = nc.dram_tensor("swi", [P, KI * 2 * H], BF16, kind="ExternalInput").ap()
    swo = nc.dram_tensor("swo", [P, KI * H], BF16, kind="ExternalInput").ap()
    ye = nc.dram_tensor("ye", [H, m_pad], BF16, kind="ExternalOutput").ap()
    ys = nc.dram_tensor("ys", [H, TS], BF16, kind="ExternalOutput").ap()

    # Raw (un-tracked, never-written) SBUF region for the HAM warm-up
    # matmuls: no memset and no dependencies, so the PE starts warming the
    # clock gate immediately after its preamble — maximum margin for the
    # free-running HAM window to flip before the real matmuls begin.
    # The garbage values it computes go to a scratch DRAM sink.
    warm_sb = nc.alloc_sbuf_tensor("warm_raw", [P, MAXN], BF16).ap()

    with ExitStack() as ctx:
        tc = ctx.enter_context(tile.TileContext(nc))
        wpool = ctx.enter_context(tc.tile_pool(name="weights", bufs=1))
        apool = ctx.enter_context(tc.tile_pool(name="act", bufs=3))
        spool = ctx.enter_context(tc.tile_pool(name="silu", bufs=4))
        # yt slots are freed by their output DMA's completion; 3 bufs made a
        # Wo-phase matmul chain wait on the SWDGE drain once per trace
        ypool = ctx.enter_context(tc.tile_pool(name="y", bufs=6))
        # all 8 PSUM banks in one pool; the warm-up tiles share the "ps" tag
        # so their 2 banks recycle into the working set after the lead-in
        psum = ctx.enter_context(tc.tile_pool(name="psum", bufs=8, space="PSUM"))

        # Warm-up matmuls on scratch data fill the otherwise-idle PE during
        # the DMA lead-in: the HAM clock gate sees a busy window and
        # un-throttles before the real matmuls start.
        warm_sink = nc.dram_tensor("warm_sink", [P, MAXN], F32).ap()
        wps = [psum.tile([P, MAXN], F32, tag="ps", name=f"wps{i}")
               for i in range(2)]
        for i in range(N_WARM):
            nc.tensor.matmul(wps[i % 2], lhsT=warm_sb[:, :P], rhs=warm_sb[:],
                             start=True, stop=True)
        warm_out = ypool.tile([P, MAXN], F32, tag="y", name="warm_out")
        nc.vector.tensor_copy(warm_out[:], wps[1])
        nc.gpsimd.dma_start(warm_sink[:], warm_out[:])

        # shared-expert job first: its x slice and the first shared-Wi pair
        # are the smallest loads, so the PE starts earliest and the (larger)
        # expert weight/token streams hide behind the shared job's compute.
        # Each HWDGE queue moves ~160 GB/s, so the first chain's inputs are
        # split across queues (xs in halves, first swi pair in halves) —
        # they land ~1.7us earlier than single-queue transfers would.
        # Issue order follows the first chain's consumption order: the ps_a
        # chain reads swiP0-A + xs k-tiles 0..5, then ps_b reads swiP0-B —
        # so (xs lo-half, A0, xs hi-half, B0) lets the chain start as soon
        # as the first pieces land instead of after the whole lead set.
        xs_t = wpool.tile([P, KH, TS], BF16, tag="xs", name="xs")
        xs3 = xs.rearrange("p (o f) -> p o f", o=KH)
        swi_p = [wpool.tile([P, 2, H], BF16, tag=f"swiP{ft}", name=f"swiP{ft}")
                 for ft in range(KI)]

        def swi_src(ft):
            return (swi[:, ft * 2 * H:(ft + 1) * 2 * H]
                    .rearrange("p (s f) -> p s f", s=2))

        # (Splitting EVERY pair into halves was tried and hurt: the extra
        # early DMAs compete with the critical xs hi-half and stalled the
        # first chain mid-stream for ~2.8us.)
        nc.sync.dma_start(xs_t[:, :KH // 2], xs3[:, :KH // 2])
        nc.sync.dma_start(swi_p[0][:, 0], swi_src(0)[:, 0])
        nc.sync.dma_start(xs_t[:, KH // 2:], xs3[:, KH // 2:])
        nc.sync.dma_start(swi_p[0][:, 1], swi_src(0)[:, 1])
        for ft in range(1, KI):
            nc.sync.dma_start(swi_p[ft][:], swi_src(ft))

        named = {}  # late-bound tiles for the expert job + shared Wo

        # accessors: (ft|ht, kt) -> lhsT AP; x: (kt) -> rhs AP
        sh = dict(
            x=lambda kt: xs_t[:, kt],
            wa=lambda ft, kt: swi_p[ft][:, 0, kt * P:(kt + 1) * P],
            wb=lambda ft, kt: swi_p[ft][:, 1, kt * P:(kt + 1) * P],
            wo=lambda ht, kt: named["swo"][:, kt, ht * P:(ht + 1) * P],
        )
        ex = dict(
            x=lambda kt: named["xe"][:, kt],
            wa=lambda ft, kt: named["wi"][:, kt, ft * P:(ft + 1) * P],
            wb=lambda ft, kt: named["wi"][:, kt, I + ft * P:I + (ft + 1) * P],
            wo=lambda ht, kt: named["wo"][:, kt, ht * P:(ht + 1) * P],
        )

        # (accessors, y_dram, chunk_off, chunk_sz, silu_on_first)
        chunks = []
        for acc, yd, m, sfirst in ((sh, ys, TS, True), (ex, ye, m_pad, False)):
            off = 0
            for sz in _chunk_sizes(m):
                chunks.append((acc, yd, off, sz, sfirst))
                off += sz

        def emit_wi(c):
            acc, yd, off, sz, sfirst = chunks[c]
            act = apool.tile([P, KI, MAXN], BF16, tag="act", name="act")[:, :, :sz]
            for ft in range(KI):
                ps_a = psum.tile([P, MAXN], F32, tag="ps", name="ps_a")[:, :sz]
                for kt in range(KH):
                    nc.tensor.matmul(ps_a, lhsT=acc["wa"](ft, kt),
                                     rhs=acc["x"](kt)[:, off:off + sz],
                                     start=(kt == 0), stop=(kt == KH - 1))
                ps_b = psum.tile([P, MAXN], F32, tag="ps", name="ps_b")[:, :sz]
                for kt in range(KH):
                    nc.tensor.matmul(ps_b, lhsT=acc["wb"](ft, kt),
                                     rhs=acc["x"](kt)[:, off:off + sz],
                                     start=(kt == 0), stop=(kt == KH - 1))
                sl = spool.tile([P, MAXN], F32, tag="silu", name="sl")[:, :sz]
                ps_s, ps_m = (ps_a, ps_b) if sfirst else (ps_b, ps_a)
                if USE_SILU:
                    # act = silu(s) * m: one ACT op + one DVE mul; PSUM banks
                    # are freed one op earlier than the sigmoid+2-mul form
                    nc.scalar.activation(sl, ps_s,
                                         mybir.ActivationFunctionType.Silu)
                    nc.vector.tensor_mul(act[:, ft, :], sl, ps_m)
                else:
                    # CoreSim fallback: silu(s) = s * sigmoid(s)
                    tmp = spool.tile([P, MAXN], F32, tag="silu2",
                                     name="tmp")[:, :sz]
                    nc.scalar.activation(sl, ps_s,
                                         mybir.ActivationFunctionType.Sigmoid)
                    nc.vector.tensor_mul(tmp, sl, ps_s)
                    nc.vector.tensor_mul(act[:, ft, :], tmp, ps_m)
            return act

        def emit_wo(c, act, last=False):
            acc, yd, off, sz, sfirst = chunks[c]
            ydst = yd.rearrange("(o p) m -> p o m", p=P)
            for ht in range(KH):
                ps_y = psum.tile([P, MAXN], F32, tag="ps", name="ps_y")[:, :sz]
                for kt in range(KI):
                    nc.tensor.matmul(ps_y, lhsT=acc["wo"](ht, kt),
                                     rhs=act[:, kt, :],
                                     start=(kt == 0), stop=(kt == KI - 1))
                yt = ypool.tile([P, MAXN], BF16, tag="y", name="yt")[:, :sz]
                # These copies interleave (software-pipelined) with the NEXT
                # chunk's silu/mul PSUM consumers on strict-FIFO engine
                # queues — a copy queued ahead of a silu delayed the PSUM
                # bank hand-back every 4th f-tile (432ns stall each).
                # Alternating ACT/DVE halves each queue's copy burden; the
                # last chunk's copies all go to the (by-then idle) DVE.
                if last or ht % 2:
                    nc.vector.tensor_copy(yt, ps_y)
                else:
                    nc.scalar.copy(yt, ps_y)
                dma_eng = nc.sync if last else nc.gpsimd
                dma_eng.dma_start(ydst[:, ht, off:off + sz], yt)

        # software pipeline: Wi(c+1) is emitted before Wo(c) so the PE always
        # has independent matmul work while ACT/DVE finish chunk c's SwiGLU.
        # Remaining loads are emitted right after the first chunk's Wi.
        n = len(chunks)
        acts = [None] * n
        acts[0] = emit_wi(0)
        t = wpool.tile([P, KI, H], BF16, tag="swo", name="swo")
        nc.sync.dma_start(t[:], swo.rearrange("p (o f) -> p o f", o=KI))
        named["swo"] = t
        t = wpool.tile([P, KH, m_pad], BF16, tag="xe", name="xe")
        nc.sync.dma_start(t[:], xe.rearrange("p (o f) -> p o f", o=KH))
        named["xe"] = t
        t = wpool.tile([P, KH, I2], BF16, tag="wi", name="wi")
        nc.sync.dma_start(t[:], wi.rearrange("p (o f) -> p o f", o=KH))
        named["wi"] = t
        t = wpool.tile([P, KI, H], BF16, tag="wo", name="wo")
        nc.sync.dma_start(t[:], wo.rearrange("p (o f) -> p o f", o=KI))
        named["wo"] = t
        # shared Wo BEFORE the first expert Wi: its weights are already
        # resident, so the PE never head-of-line blocks on the expert weight
        # stream (an idle window >3.4us would re-throttle the HAM clock gate)
        emit_wo(0, acts[0])
        if n > 1:
            acts[1] = emit_wi(1)
            for c in range(2, n):
                acts[c] = emit_wi(c)
                emit_wo(c - 1, acts[c - 1])
            emit_wo(n - 1, acts[n - 1], last=True)

    nc.compile()
    return nc


def _pack_rows(a: np.ndarray) -> np.ndarray:
    """(ktiles*P, F) f32/bf16 -> packed [P, ktiles*F] (row p holds each
    k-tile's row p, concatenated k-tile-major)."""
    kt = a.shape[0] // P
    return np.ascontiguousarray(
        a.reshape(kt, P, a.shape[1]).transpose(1, 0, 2).reshape(P, -1))


def _pack_swi(swiT: np.ndarray) -> np.ndarray:
    """shared_Wi.T (H, 2I) -> [P, KI*2*H]: per f-tile pair (A_ft, B_ft),
    each [P, H] with swi[p, ft, s, kt*P + c] = swiT[kt*P + p, (s*I) + ft*P + c]."""
    # (H, 2I) -> (KH, P, 2, KI, P): kt, p, s(half), ft, c
    r = swiT.reshape(KH, P, 2, KI, P)
    # -> (p, ft, s, kt, c)
    return np.ascontiguousarray(
        r.transpose(1, 3, 2, 0, 4).reshape(P, -1))


def _route(x, gate_w, correction_bias):
    logits = 1.0 / (1.0 + np.exp(-(x @ gate_w.T), dtype=np.float32))  # (T, E)
    sel = logits + correction_bias[None, :]
    order = np.argsort(-sel, axis=1, kind="stable")[:, :TOPK]  # ties -> low index
    w = np.take_along_axis(logits, order, axis=1)
    w = (w / w.sum(axis=1, keepdims=True)).astype(np.float32)
    return order, w


def kernel(**inputs) -> np.ndarray:
    x = np.asarray(inputs["x"], np.float32)
    gate_w = np.asarray(inputs["gate_w"], np.float32)
    bias = np.asarray(inputs["correction_bias"], np.float32)
    Wi = np.asarray(inputs["Wi"], np.float32)
    Wo = np.asarray(inputs["Wo"], np.float32)
    shared_Wi = np.asarray(inputs["shared_Wi"], np.float32)
    shared_Wo = np.asarray(inputs["shared_Wo"], np.float32)

    order, w = _route(x, gate_w, bias)

    idx_per_e, cw_per_e = [], []
    for e in range(E):
        mask = order == e  # (T, K)
        tok = mask.any(axis=1)
        rows = np.nonzero(tok)[0]
        kpos = np.argmax(mask[rows], axis=1)
        idx_per_e.append(rows)
        cw_per_e.append(w[rows, kpos].astype(np.float32))

    mx = max(len(r) for r in idx_per_e)
    m_pad = max(64, mx + (mx & 1))  # exact capacity, kept even for alignment

    bf = ml_dtypes.bfloat16
    xT = np.ascontiguousarray(x.T).astype(bf)        # (H, T) bf16
    swi_packed = _pack_swi(shared_Wi.T.astype(bf))   # [P, KI*2*H]
    swo_packed = _pack_rows(
        np.ascontiguousarray(shared_Wo.T).astype(bf))  # [P, KI*H]

    in_maps = []
    for c in range(N_CORES):
        rows = idx_per_e[c]
        xe = np.zeros((H, m_pad), bf)
        xe[:, :len(rows)] = xT[:, rows]
        in_maps.append({
            "xe": _pack_rows(xe),                            # [P, KH*m_pad]
            "wi": _pack_rows(Wi[c].astype(bf)),              # [P, KH*2I]
            "wo": _pack_rows(Wo[c].astype(bf)),              # [P, KI*H]
            "xs": _pack_rows(
                np.ascontiguousarray(xT[:, c * TS:(c + 1) * TS])),
            "swi": swi_packed,
            "swo": swo_packed,
        })

    if m_pad not in _BUILD_CACHE:
        _BUILD_CACHE[m_pad] = _build(m_pad)
    nc = _BUILD_CACHE[m_pad]

    _ensure_axon_ntff_hook()
    res = run_bass_kernel_spmd(nc, in_maps, list(range(N_CORES)))
    global LAST_RESULTS
    LAST_RESULTS = res

    out = np.zeros((T, H), np.float32)
    for c in range(N_CORES):
        r = res.results[c]
        out[c * TS:(c + 1) * TS] += r["ys"].astype(np.float32).T
        rows = idx_per_e[c]
        if len(rows):
            out[rows] += (r["ye"][:, :len(rows)].astype(np.float32).T
                          * cw_per_e[c][:, None])
    return out
